# revision 1
# baseline (speedup 1.0000x reference)
"""Trainium2 Bass kernel for nn_Attention (B=4, N=2048, DIM=512, H=8).

Sharding: 8 cores = (batch b, seq-half s). Each core computes attention
outputs for queries [s*1024, (s+1)*1024) of batch b, all 8 heads, plus
the output projection for those rows. Outputs are disjoint -> host
gather is a pure concatenation (no reduction). Keys are permuted per
core (own seq-half first) so the query chunk is always columns [0, NQ)
of the permuted x.T; attention is permutation-invariant over keys.

Per-core dataflow (layouts chosen so no on-device transposes of large
tensors are needed):
  q_T [512,1024]  = (SCALE*wq) @ x_chunk.T    (features x queries)
  k_T [512,2048]  = wk @ x.T                  (features x keys)
  v   [2048, 512] = x @ wv.T                  (keys x features)
  per head-pair, seq-half, key-tile kt:
       scores_T[k, (h0 q | h1 q)] = k_h @ q_h.T   (K=64 row-packed pair)
       p = exp(scores_T) * exp(mask).T         (mask add via exp-multiply)
       U.T[d,q] += v_h.T @ p   (PSUM accum over kt, heads col-packed M=64)
       sums[q]  += p.T @ ones  (N=1 matmuls, queries on partitions)
  Uhat = U * (1/sums broadcast via PE transpose + K=2 indicator matmul)
  out[q,:] = Uhat.T @ proj_w.T + bias
"""
import functools
import numpy as np
import ml_dtypes
from contextlib import ExitStack

import concourse.bass as bass
import concourse.tile as tile
from concourse import bacc, mybir
from concourse.bass_utils import run_bass_kernel_spmd

F32 = mybir.dt.float32
F32R = mybir.dt.float32r
BF16 = mybir.dt.bfloat16
AF = mybir.ActivationFunctionType

B, N, DIM, H, D = 4, 2048, 512, 8, 64
SCALE = D ** -0.5
NQ = N // 2          # queries per core
NKT = N // 128       # key tiles (16)
NCORES = 8


def build(dbg=False):
    nc = bacc.Bacc("TRN2", target_bir_lowering=False, debug=False,
                   num_devices=NCORES)
    xT = nc.dram_tensor("xT", [DIM, N], F32R, kind="ExternalInput").ap()
    wqT = nc.dram_tensor("wqT", [DIM, DIM], F32R, kind="ExternalInput").ap()
    wkT = nc.dram_tensor("wkT", [DIM, DIM], F32R, kind="ExternalInput").ap()
    wvT = nc.dram_tensor("wvT", [DIM, DIM], F32R, kind="ExternalInput").ap()
    projT = nc.dram_tensor("projT", [DIM, DIM], F32R, kind="ExternalInput").ap()
    biasb = nc.dram_tensor("biasb", [128, DIM], F32, kind="ExternalInput").ap()
    expmT = nc.dram_tensor("expmT", [N, NQ], BF16, kind="ExternalInput").ap()
    indD = nc.dram_tensor("indD", [2, 128], F32, kind="ExternalInput").ap()
    out = nc.dram_tensor("out", [NQ, DIM], F32, kind="ExternalOutput").ap()

    with tile.TileContext(nc) as tc, ExitStack() as ctx:
        # ---- SBUF pools ----
        wp = ctx.enter_context(tc.tile_pool(name="wp", bufs=1))
        kv = ctx.enter_context(tc.tile_pool(name="kv", bufs=1))
        small = ctx.enter_context(tc.tile_pool(name="small", bufs=2))
        osb = ctx.enter_context(tc.tile_pool(name="osb", bufs=2))
        # ---- PSUM pools: 4 + 2 + 1 + 1 = 8 banks ----
        ps_stage = ctx.enter_context(
            tc.tile_pool(name="ps_stage", bufs=2, space="PSUM"))   # 2x2 banks
        ps_out = ctx.enter_context(
            tc.tile_pool(name="ps_out", bufs=2, space="PSUM"))     # 2x1 bank
        ps_sums = ctx.enter_context(
            tc.tile_pool(name="ps_sums", bufs=1, space="PSUM"))    # 1 bank

        # ---- constants ----
        ones_bf = wp.tile([128, 1], BF16, name="ones_bf", tag="ones_bf")
        nc.vector.memset(ones_bf[:], 1.0)

        # ---- persistent loads ----
        pj_sb = [wp.tile([128, DIM], F32R, name=f"pj{kc}", tag=f"pj{kc}")
                 for kc in range(4)]
        for kc in range(4):
            nc.scalar.dma_start(pj_sb[kc][:], projT[kc * 128:(kc + 1) * 128, :])
        bias_sb = wp.tile([128, DIM], F32, name="bias_sb", tag="bias_sb")
        nc.sync.dma_start(bias_sb[:], biasb[:])

        q_sb = [kv.tile([128, NQ], F32R, name=f"q{m}", tag=f"q{m}")
                for m in range(4)]
        k_sb = [kv.tile([128, N], F32R, name=f"k{m}", tag=f"k{m}")
                for m in range(4)]
        v_sb = [kv.tile([128, DIM], BF16, name=f"v{kt}", tag=f"v{kt}")
                for kt in range(NKT)]
        em_sb = [kv.tile([128, NQ], BF16, name=f"em{kt}", tag=f"em{kt}")
                 for kt in range(NKT)]
        for kt in range(NKT):
            nc.gpsimd.dma_start(em_sb[kt][:], expmT[kt * 128:(kt + 1) * 128, :])
        uhat = [kv.tile([128, NQ], F32R, name=f"uh{p}", tag=f"uh{p}")
                for p in range(4)]

        # ---- phase 1: projections ----
        with tc.tile_pool(name="xp", bufs=1) as xp:
            x_sb = [xp.tile([128, N], F32R, name=f"x{kc}", tag=f"x{kc}")
                    for kc in range(4)]
            wq_sb = [xp.tile([128, DIM], F32R, name=f"wq{kc}", tag=f"wq{kc}")
                     for kc in range(4)]
            wk_sb = [xp.tile([128, DIM], F32R, name=f"wk{kc}", tag=f"wk{kc}")
                     for kc in range(4)]
            wv_sb = [xp.tile([128, DIM], F32R, name=f"wv{kc}", tag=f"wv{kc}")
                     for kc in range(4)]
            for kc in range(4):
                sl = slice(kc * 128, (kc + 1) * 128)
                (nc.sync if kc % 2 == 0 else nc.scalar).dma_start(
                    x_sb[kc][:], xT[sl, :])
            for kc in range(4):
                sl = slice(kc * 128, (kc + 1) * 128)
                nc.scalar.dma_start(wq_sb[kc][:], wqT[sl, :])
                nc.sync.dma_start(wk_sb[kc][:], wkT[sl, :])
                nc.scalar.dma_start(wv_sb[kc][:], wvT[sl, :])

            # q_T [512, 1024] (queries = cols 0:NQ of permuted xT)
            for m in range(4):
                ms = slice(m * 128, (m + 1) * 128)
                ps = ps_stage.tile([128, NQ], F32, name=f"psq{m}", tag="stage")
                for c in range(2):
                    cs = slice(c * 512, (c + 1) * 512)
                    for kc in range(4):
                        nc.tensor.matmul(ps[:, cs], wq_sb[kc][:, ms],
                                         x_sb[kc][:, cs],
                                         start=(kc == 0), stop=(kc == 3))
                nc.vector.tensor_copy(q_sb[m][:], ps[:])

            # k_T [512, 2048]
            for m in range(4):
                ms = slice(m * 128, (m + 1) * 128)
                for half in range(2):
                    ps = ps_stage.tile([128, NQ], F32, name=f"psk{m}_{half}",
                                       tag="stage")
                    for c2 in range(2):
                        cs_o = slice(c2 * 512, (c2 + 1) * 512)
                        cs_x = slice(half * 1024 + c2 * 512,
                                     half * 1024 + (c2 + 1) * 512)
                        for kc in range(4):
                            nc.tensor.matmul(ps[:, cs_o], wk_sb[kc][:, ms],
                                             x_sb[kc][:, cs_x],
                                             start=(kc == 0), stop=(kc == 3))
                    nc.vector.tensor_copy(
                        k_sb[m][:, half * 1024:(half + 1) * 1024], ps[:])

            # v [2048, 512]
            for kt in range(NKT):
                ks = slice(kt * 128, (kt + 1) * 128)
                ps = ps_out.tile([128, DIM], F32, name=f"psv{kt}", tag="out")
                for kc in range(4):
                    nc.tensor.matmul(ps[:], x_sb[kc][:, ks], wv_sb[kc][:],
                                     start=(kc == 0), stop=(kc == 3))
                nc.vector.tensor_copy(v_sb[kt][:], ps[:])

        # ---- phase 2: attention ----
        with tc.tile_pool(name="praw", bufs=3) as praw_p, \
             tc.tile_pool(name="phat", bufs=6) as phat_p:
            pending_norm = [None]

            def emit_norm():
                if pending_norm[0] is None:
                    return
                pr, hf, o_ps, srow = pending_norm[0]
                pending_norm[0] = None
                hq2 = slice(hf * 512, (hf + 1) * 512)
                rrow1 = small.tile([1, 1024], F32,
                                   name=f"rw1{pr}_{hf}", tag="rw1")
                nc.vector.reciprocal_approx_fast(rrow1[:], srow[:])
                bc_sb = small.tile([128, 512], F32,
                                   name=f"bs{pr}_{hf}", tag="bs")
                for hi2 in range(2):
                    srcb = rrow1[0:1, hi2 * 512:(hi2 + 1) * 512].rearrange(
                        "p (o f) -> p o f", o=1).broadcast_to([1, 64, 512])
                    nc.sync.dma_start(bc_sb[hi2 * 64:(hi2 + 1) * 64, :], srcb)
                nc.vector.tensor_mul(uhat[pr][:, hq2], o_ps[:], bc_sb[:])

            for pair in range(4):
                for half in range(2):
                    hq = slice(half * 512, (half + 1) * 512)
                    out_ps = ps_out.tile([128, 512], F32,
                                         name=f"o{pair}_{half}", tag="out")
                    sums_ps = ps_sums.tile([1, 1024], F32,
                                           name=f"s{pair}_{half}", tag="sums")
                    for kt in range(NKT):
                        if kt == 4:
                            emit_norm()
                        kts = slice(kt * 128, (kt + 1) * 128)
                        st = ps_stage.tile([128, 1024], F32,
                                           name=f"st{pair}_{half}_{kt}",
                                           tag="stage")
                        # scores: both heads adjacent -> row-packed pair
                        for hi in range(2):
                            po = hi * 64
                            pos = slice(po, po + 64)
                            nc.tensor.matmul(
                                st[:, hi * 512:(hi + 1) * 512],
                                k_sb[pair][pos, kts], q_sb[pair][pos, hq],
                                start=True, stop=True, tile_position=(po, 0))
                        praw = praw_p.tile([128, 1024], BF16,
                                           name=f"pr{pair}_{half}_{kt}",
                                           tag="pr")
                        nc.scalar.activation(praw[:], st[:], AF.Exp)
                        phat = phat_p.tile([128, 1024], BF16,
                                           name=f"ph{pair}_{half}_{kt}",
                                           tag="ph")
                        em2 = em_sb[kt][:, hq].rearrange(
                            "p (o f) -> p o f", o=1).broadcast_to([128, 2, 512])
                        nc.vector.tensor_mul(
                            phat[:].rearrange("p (t f) -> p t f", t=2),
                            praw[:].rearrange("p (t f) -> p t f", t=2), em2)
                        # p@v: col-packed pair
                        for hi in range(2):
                            h = 2 * pair + hi
                            po = hi * 64
                            pos = slice(po, po + 64)
                            nc.tensor.matmul(
                                out_ps[pos, :],
                                v_sb[kt][:, h * 64:(h + 1) * 64],
                                phat[:, hi * 512:(hi + 1) * 512],
                                start=(kt == 0), stop=(kt == NKT - 1),
                                skip_group_check=(hi == 1),
                                tile_position=(0, po))
                        # sums: wide streaming matmuls, one per head
                        for hi in range(2):
                            nc.tensor.matmul(
                                sums_ps[0:1, hi * 512:(hi + 1) * 512],
                                ones_bf[:],
                                phat[:, hi * 512:(hi + 1) * 512],
                                start=(kt == 0), stop=(kt == NKT - 1))

                    # evacuate sums now (frees the bank); defer the rest
                    srow = small.tile([1, 1024], F32,
                                      name=f"sr{pair}_{half}", tag="sr")
                    nc.vector.tensor_copy(srow[:], sums_ps[0:1, :])
                    pending_norm[0] = (pair, half, out_ps, srow)
            emit_norm()

        # ---- phase 3: output projection ----
        for m in range(8):
            ms = slice(m * 128, (m + 1) * 128)
            pp = ps_stage.tile([128, DIM], F32, name=f"pp{m}", tag="stage")
            for kc in range(4):
                nc.tensor.matmul(pp[:], uhat[kc][:, ms], pj_sb[kc][:],
                                 start=(kc == 0), stop=(kc == 3))
            ob = osb.tile([128, DIM], F32, name=f"ob{m}", tag="ob")
            nc.vector.tensor_add(ob[:], pp[:], bias_sb[:])
            nc.sync.dma_start(out[ms, :], ob[:])

    nc.compile()
    return nc


@functools.lru_cache(maxsize=1)
def _get_nc():
    return build()


def _prep_inputs(x, attn_mask, qkv_w, proj_w, proj_b):
    x = np.asarray(x, dtype=np.float32)
    mask = np.asarray(attn_mask, dtype=np.float32).reshape(N, N)
    qkv_w = np.asarray(qkv_w, dtype=np.float32)
    proj_w = np.asarray(proj_w, dtype=np.float32)
    proj_b = np.asarray(proj_b, dtype=np.float32)

    wqT = np.ascontiguousarray((qkv_w[0:DIM] * SCALE).T)
    wkT = np.ascontiguousarray(qkv_w[DIM:2 * DIM].T)
    wvT = np.ascontiguousarray(qkv_w[2 * DIM:3 * DIM].T)
    projT = np.ascontiguousarray(proj_w.T)
    biasb = np.tile(proj_b, (128, 1))
    indmat = np.zeros((2, 128), dtype=np.float32)
    indmat[0, 0:64] = 1.0
    indmat[1, 64:128] = 1.0

    expm = np.exp(mask)
    # per-core key permutation: own seq-half first, other half second, so
    # the query chunk is always columns [0, NQ) of the permuted x.T
    xTs = {}
    emTs = {}
    for s in range(2):
        o = 1 - s
        emT = np.ascontiguousarray(expm[s * NQ:(s + 1) * NQ, :].T)  # [keys, q]
        emTs[s] = np.concatenate(
            [emT[s * NQ:(s + 1) * NQ], emT[o * NQ:(o + 1) * NQ]], axis=0
        ).astype(ml_dtypes.bfloat16)
        for b in range(B):
            xTb = x[b].T  # [DIM, N]
            xTs[(b, s)] = np.ascontiguousarray(np.concatenate(
                [xTb[:, s * NQ:(s + 1) * NQ], xTb[:, o * NQ:(o + 1) * NQ]],
                axis=1))

    in_maps = []
    for c in range(NCORES):
        b, s = c // 2, c % 2
        in_maps.append({
            "xT": xTs[(b, s)],
            "wqT": wqT, "wkT": wkT, "wvT": wvT, "projT": projT,
            "biasb": biasb, "expmT": emTs[s], "indD": indmat,
        })
    return in_maps


def run(inputs, trace=False, tmpdir=None):
    nc = _get_nc()
    in_maps = _prep_inputs(**inputs)
    res = run_bass_kernel_spmd(nc, in_maps, core_ids=list(range(NCORES)),
                               trace=trace, tmpdir=tmpdir)
    full = np.empty((B, N, DIM), dtype=np.float32)
    for c in range(NCORES):
        b, s = c // 2, c % 2
        full[b, s * NQ:(s + 1) * NQ, :] = res.results[c]["out"]
    return full, res


def kernel(**inputs) -> np.ndarray:
    return run(inputs)[0]



# revision 4
# speedup vs baseline: 1.1901x; 1.1901x over previous
"""Trainium2 Bass kernel for nn_Attention (B=4, N=2048, DIM=512, H=8).

Sharding: 8 cores = (batch b, seq-half s). Each core computes attention
outputs for queries [s*1024, (s+1)*1024) of batch b, all 8 heads, plus
the output projection for those rows. Outputs are disjoint -> host
gather is a pure concatenation (no reduction). Keys are permuted per
core (own seq-half first) so the query chunk is always columns [0, NQ)
of the permuted x.T; attention is permutation-invariant over keys.

Per-core dataflow (all matmul operands bf16, PSUM accumulation f32):
  q_T [512,1024]  = (SCALE*wq) @ x_chunk.T    (features x queries)
  k_T [512,2048]  = wk @ x.T                  (features x keys)
  v_aug [2048,520]= x @ wv.T per head + ones col (65 cols per head)
  per head-pair, query-half, key-tile kt:
       scores_T[k, (h0 q | h1 q)] = k_h @ q_h.T  (K=64 row-packed pair)
       p = exp(scores_T) * exp(mask).T        (mask add via exp-multiply)
       pv_h[0:64,q] += v_h.T @ p_h  (PSUM accum over kt; 65-col
       pv_h[64, q]  += ones . p_h    stationary gives sums row free)
  norm: DMA-broadcast row 64 -> [128,512], reciprocal on DVE,
        uhat = pv * recip on Pool engine (keeps DVE queue clear)
  out[q,:] = uhat.T @ proj_w.T + bias  (emitted per query-half so the
        projection/output DMA of half 0 overlaps half-1 attention)
"""
import functools
import numpy as np
import ml_dtypes
from contextlib import ExitStack

import concourse.bass as bass
import concourse.tile as tile
from concourse import bacc, mybir
from concourse.bass_utils import run_bass_kernel_spmd

F32 = mybir.dt.float32
BF16 = mybir.dt.bfloat16
AF = mybir.ActivationFunctionType

B, N, DIM, H, D = 4, 2048, 512, 8, 64
SCALE = D ** -0.5
NQ = N // 2          # queries per core
NKT = N // 128       # key tiles (16)
NCORES = 8


def build(dbg=False):
    nc = bacc.Bacc("TRN2", target_bir_lowering=False, debug=False,
                   num_devices=NCORES)
    xT = nc.dram_tensor("xT", [DIM, N], BF16, kind="ExternalInput").ap()
    wqT = nc.dram_tensor("wqT", [DIM, DIM], BF16, kind="ExternalInput").ap()
    wkT = nc.dram_tensor("wkT", [DIM, DIM], BF16, kind="ExternalInput").ap()
    wvT = nc.dram_tensor("wvT", [DIM, DIM], BF16, kind="ExternalInput").ap()
    projT = nc.dram_tensor("projT", [DIM, DIM], BF16, kind="ExternalInput").ap()
    biasb = nc.dram_tensor("biasb", [128, DIM], F32, kind="ExternalInput").ap()
    expmT = nc.dram_tensor("expmT", [N, NQ], BF16, kind="ExternalInput").ap()
    out = nc.dram_tensor("out", [NQ, DIM], F32, kind="ExternalOutput").ap()

    with tile.TileContext(nc) as tc, ExitStack() as ctx:
        # ---- SBUF pools ----
        wp = ctx.enter_context(tc.tile_pool(name="wp", bufs=1))
        kv = ctx.enter_context(tc.tile_pool(name="kv", bufs=1))
        small = ctx.enter_context(tc.tile_pool(name="small", bufs=2))
        osb = ctx.enter_context(tc.tile_pool(name="osb", bufs=2))
        # ---- PSUM pools: 2x2 + 4x1 = 8 banks ----
        ps_stage = ctx.enter_context(
            tc.tile_pool(name="ps_stage", bufs=2, space="PSUM"))   # 2x2 banks
        ps_pv = ctx.enter_context(
            tc.tile_pool(name="ps_pv", bufs=4, space="PSUM"))      # 4x1 bank

        # ---- persistent tiles ----
        pj_sb = [wp.tile([128, DIM], BF16, name=f"pj{kc}", tag=f"pj{kc}")
                 for kc in range(4)]
        bias_sb = wp.tile([128, DIM], F32, name="bias_sb", tag="bias_sb")
        q_sb = [kv.tile([128, NQ], BF16, name=f"q{m}", tag=f"q{m}")
                for m in range(4)]
        k_sb = [kv.tile([128, N], BF16, name=f"k{m}", tag=f"k{m}")
                for m in range(4)]
        # per-head 65th column is ones -> sums row lands in PSUM row 64
        v_sb = [kv.tile([128, 8 * 65], BF16, name=f"v{kt}", tag=f"v{kt}")
                for kt in range(NKT)]
        em_sb = [kv.tile([128, NQ], BF16, name=f"em{kt}", tag=f"em{kt}")
                 for kt in range(NKT)]
        uhat = [kv.tile([128, NQ], BF16, name=f"uh{p}", tag=f"uh{p}")
                for p in range(4)]

        for kt in range(NKT):
            nc.gpsimd.memset(
                v_sb[kt][:].rearrange("p (h c) -> p h c", h=8)[:, :, 64:65],
                1.0)

        # ---- phase 1: projections (DMAs ordered so compute starts early;
        #      em mask tiles queue last on the same queues) ----
        with tc.tile_pool(name="xp", bufs=1) as xp:
            x_sb = [xp.tile([128, N], BF16, name=f"x{kc}", tag=f"x{kc}")
                    for kc in range(4)]
            wq_sb = [xp.tile([128, DIM], BF16, name=f"wq{kc}", tag=f"wq{kc}")
                     for kc in range(4)]
            wk_sb = [xp.tile([128, DIM], BF16, name=f"wk{kc}", tag=f"wk{kc}")
                     for kc in range(4)]
            wv_sb = [xp.tile([128, DIM], BF16, name=f"wv{kc}", tag=f"wv{kc}")
                     for kc in range(4)]
            # sync queue: x0, x2, wk, wv, em0-7
            # scalar queue: wq, x1, x3, proj, bias, em8-15
            for kc in range(4):
                sl = slice(kc * 128, (kc + 1) * 128)
                nc.scalar.dma_start(wq_sb[kc][:], wqT[sl, :])
            nc.sync.dma_start(x_sb[0][:], xT[0:128, :])
            nc.scalar.dma_start(x_sb[1][:], xT[128:256, :])
            nc.sync.dma_start(x_sb[2][:], xT[256:384, :])
            nc.scalar.dma_start(x_sb[3][:], xT[384:512, :])
            for kc in range(4):
                sl = slice(kc * 128, (kc + 1) * 128)
                nc.sync.dma_start(wk_sb[kc][:], wkT[sl, :])
            for kc in range(4):
                sl = slice(kc * 128, (kc + 1) * 128)
                nc.sync.dma_start(wv_sb[kc][:], wvT[sl, :])
            for kc in range(4):
                sl = slice(kc * 128, (kc + 1) * 128)
                nc.scalar.dma_start(pj_sb[kc][:], projT[sl, :])
            nc.scalar.dma_start(bias_sb[:], biasb[:])
            for kt in range(8):
                nc.sync.dma_start(em_sb[kt][:],
                                  expmT[kt * 128:(kt + 1) * 128, :])
            for kt in range(8, NKT):
                nc.scalar.dma_start(em_sb[kt][:],
                                    expmT[kt * 128:(kt + 1) * 128, :])

            # q_T [512, 1024] (queries = cols 0:NQ of permuted xT)
            for m in range(4):
                ms = slice(m * 128, (m + 1) * 128)
                ps = ps_stage.tile([128, NQ], F32, name=f"psq{m}", tag="stage")
                for c in range(2):
                    cs = slice(c * 512, (c + 1) * 512)
                    for kc in range(4):
                        nc.tensor.matmul(ps[:, cs], wq_sb[kc][:, ms],
                                         x_sb[kc][:, cs],
                                         start=(kc == 0), stop=(kc == 3))
                nc.vector.tensor_copy(q_sb[m][:], ps[:])

            # k_T [512, 2048]
            for m in range(4):
                ms = slice(m * 128, (m + 1) * 128)
                for half in range(2):
                    ps = ps_stage.tile([128, NQ], F32, name=f"psk{m}_{half}",
                                       tag="stage")
                    for c2 in range(2):
                        cs_o = slice(c2 * 512, (c2 + 1) * 512)
                        cs_x = slice(half * 1024 + c2 * 512,
                                     half * 1024 + (c2 + 1) * 512)
                        for kc in range(4):
                            nc.tensor.matmul(ps[:, cs_o], wk_sb[kc][:, ms],
                                             x_sb[kc][:, cs_x],
                                             start=(kc == 0), stop=(kc == 3))
                    nc.vector.tensor_copy(
                        k_sb[m][:, half * 1024:(half + 1) * 1024], ps[:])

            # v [2048, 512] -> 65-col head blocks of v_sb
            for kt in range(NKT):
                ks = slice(kt * 128, (kt + 1) * 128)
                ps = ps_pv.tile([128, DIM], F32, name=f"psv{kt}", tag="pv")
                for kc in range(4):
                    nc.tensor.matmul(ps[:], x_sb[kc][:, ks], wv_sb[kc][:],
                                     start=(kc == 0), stop=(kc == 3))
                nc.vector.tensor_copy(
                    v_sb[kt][:].rearrange("p (h c) -> p h c", h=8)[:, :, 0:64],
                    ps[:].rearrange("p (h c) -> p h c", h=8))

        # ---- phase 2: attention (half-outer so phase 3 interleaves) ----
        with tc.tile_pool(name="praw", bufs=3) as praw_p, \
             tc.tile_pool(name="phat", bufs=6) as phat_p:
            for half in range(2):
                hq = slice(half * 512, (half + 1) * 512)
                for pair in range(4):
                    pv = [ps_pv.tile([128, 512], F32,
                                     name=f"pv{pair}_{half}_{hi}", tag="pv")
                          for hi in range(2)]
                    for kt in range(NKT):
                        kts = slice(kt * 128, (kt + 1) * 128)
                        st = ps_stage.tile([128, 1024], F32,
                                           name=f"st{pair}_{half}_{kt}",
                                           tag="stage")
                        # scores: both heads adjacent -> row-packed pair
                        for hi in range(2):
                            po = hi * 64
                            pos = slice(po, po + 64)
                            nc.tensor.matmul(
                                st[:, hi * 512:(hi + 1) * 512],
                                k_sb[pair][pos, kts], q_sb[pair][pos, hq],
                                start=True, stop=True, tile_position=(po, 0))
                        praw = praw_p.tile([128, 1024], BF16,
                                           name=f"pr{pair}_{half}_{kt}",
                                           tag="pr")
                        nc.scalar.activation(praw[:], st[:], AF.Exp)
                        phat = phat_p.tile([128, 1024], BF16,
                                           name=f"ph{pair}_{half}_{kt}",
                                           tag="ph")
                        em2 = em_sb[kt][:, hq].rearrange(
                            "p (o f) -> p o f", o=1).broadcast_to([128, 2, 512])
                        eng = nc.vector if kt % 2 == 0 else nc.gpsimd
                        eng.tensor_mul(
                            phat[:].rearrange("p (t f) -> p t f", t=2),
                            praw[:].rearrange("p (t f) -> p t f", t=2), em2)
                        # p@v per head: 65-col stationary (v | ones) puts
                        # U rows in 0:64 and the softmax sums in row 64
                        for hi in range(2):
                            h = 2 * pair + hi
                            nc.tensor.matmul(
                                pv[hi][0:65, :],
                                v_sb[kt][:, h * 65:(h + 1) * 65],
                                phat[:, hi * 512:(hi + 1) * 512],
                                start=(kt == 0), stop=(kt == NKT - 1))

                    # normalize: broadcast sums row, reciprocal, mul
                    srow = small.tile([1, 1024], F32,
                                      name=f"sr{pair}_{half}", tag="sr")
                    for hi in range(2):
                        nc.vector.tensor_copy(
                            srow[0:1, hi * 512:(hi + 1) * 512], pv[hi][64:65, :])
                    bc = small.tile([128, 512], F32,
                                    name=f"bc{pair}_{half}", tag="bc")
                    for hi in range(2):
                        srcb = srow[0:1, hi * 512:(hi + 1) * 512].rearrange(
                            "p (o f) -> p o f", o=1).broadcast_to([1, 64, 512])
                        nc.sync.dma_start(bc[hi * 64:(hi + 1) * 64, :], srcb)
                    rc = small.tile([128, 512], F32,
                                    name=f"rc{pair}_{half}", tag="rc")
                    nc.vector.reciprocal_approx_fast(rc[:], bc[:])
                    for hi in range(2):
                        nc.vector.tensor_mul(
                            uhat[pair][hi * 64:(hi + 1) * 64, hq],
                            pv[hi][0:64, :], rc[hi * 64:(hi + 1) * 64, :])

                # ---- phase 3 for this query-half ----
                for mb in range(4):
                    m = half * 4 + mb
                    ms = slice(m * 128, (m + 1) * 128)
                    pp = ps_pv.tile([128, 512], F32, name=f"pp{m}", tag="pv")
                    for kc in range(4):
                        nc.tensor.matmul(pp[:], uhat[kc][:, ms], pj_sb[kc][:],
                                         start=(kc == 0), stop=(kc == 3))
                    ob = osb.tile([128, DIM], F32, name=f"ob{m}", tag="ob")
                    nc.vector.tensor_add(ob[:], pp[:], bias_sb[:])
                    nc.scalar.dma_start(out[ms, :], ob[:])

    nc.compile()
    return nc


@functools.lru_cache(maxsize=1)
def _get_nc():
    return build()


def _prep_inputs(x, attn_mask, qkv_w, proj_w, proj_b):
    x = np.asarray(x, dtype=np.float32)
    mask = np.asarray(attn_mask, dtype=np.float32).reshape(N, N)
    qkv_w = np.asarray(qkv_w, dtype=np.float32)
    proj_w = np.asarray(proj_w, dtype=np.float32)
    proj_b = np.asarray(proj_b, dtype=np.float32)

    bf = ml_dtypes.bfloat16
    wqT = np.ascontiguousarray((qkv_w[0:DIM] * SCALE).T).astype(bf)
    wkT = np.ascontiguousarray(qkv_w[DIM:2 * DIM].T).astype(bf)
    wvT = np.ascontiguousarray(qkv_w[2 * DIM:3 * DIM].T).astype(bf)
    projT = np.ascontiguousarray(proj_w.T).astype(bf)
    biasb = np.tile(proj_b, (128, 1))

    expm = np.exp(mask)
    # per-core key permutation: own seq-half first, other half second, so
    # the query chunk is always columns [0, NQ) of the permuted x.T
    xTs = {}
    emTs = {}
    for s in range(2):
        o = 1 - s
        emT = np.ascontiguousarray(expm[s * NQ:(s + 1) * NQ, :].T)  # [keys, q]
        emTs[s] = np.concatenate(
            [emT[s * NQ:(s + 1) * NQ], emT[o * NQ:(o + 1) * NQ]], axis=0
        ).astype(bf)
        for b in range(B):
            xTb = x[b].T  # [DIM, N]
            xTs[(b, s)] = np.ascontiguousarray(np.concatenate(
                [xTb[:, s * NQ:(s + 1) * NQ], xTb[:, o * NQ:(o + 1) * NQ]],
                axis=1)).astype(bf)

    in_maps = []
    for c in range(NCORES):
        b, s = c // 2, c % 2
        in_maps.append({
            "xT": xTs[(b, s)],
            "wqT": wqT, "wkT": wkT, "wvT": wvT, "projT": projT,
            "biasb": biasb, "expmT": emTs[s],
        })
    return in_maps


def run(inputs, trace=False, tmpdir=None):
    nc = _get_nc()
    in_maps = _prep_inputs(**inputs)
    res = run_bass_kernel_spmd(nc, in_maps, core_ids=list(range(NCORES)),
                               trace=trace, tmpdir=tmpdir)
    full = np.empty((B, N, DIM), dtype=np.float32)
    for c in range(NCORES):
        b, s = c // 2, c % 2
        full[b, s * NQ:(s + 1) * NQ, :] = res.results[c]["out"]
    return full, res


def kernel(**inputs) -> np.ndarray:
    return run(inputs)[0]
